# revision 7
# baseline (speedup 1.0000x reference)
"""Causal attention kernel for TRN2, 8 NeuronCores (SPMD).

Problem:  x[4096,2048] f32; q = x@Wq.T, k = x@Wk.T (d_head=128),
          scores = q@k.T causal-masked, attn = softmax(scores),
          out = (attn @ x) @ W2.T.

Sharding: sequence-parallel over queries with stride-8 interleave:
  core c owns queries {8m+c : m=0..511}.  For key tile kt (128 keys),
  every core has exactly 512-16*kt valid queries -- a contiguous tail
  slice of its query columns -- so the SPMD program is identical on all
  cores (no dynamic control flow, no collectives) and causal work is
  perfectly balanced.

Precision: fp16 inputs for the q/k projections and the score matmul
  (fp32 PSUM accumulation), unnormalized softmax (exp without
  max-subtraction: scores are bounded ~|s|<70 for unit-normal inputs,
  safely inside fp32 exp range), attention weights in bf16 (bf16 has
  fp32 exponent range, needed for exp(s) up to ~1e28), V and W2
  matmuls in bf16/fp16, final normalization by the row-sum in fp32.

Layouts are all "transposed" (d_head / d_model on the partition dim) so
no on-device transposes are needed; the host pre-transposes inputs and
inverse-permutes the gathered outputs.
"""

import numpy as np
import ml_dtypes

import concourse.bass as bass
import concourse.bacc as bacc
import concourse.mybir as mybir
import concourse.tile as tile
from concourse.bass_utils import run_bass_kernel_spmd

N_CTX = 4096
D_MODEL = 2048
D_HEAD = 128
NCORES = 8
QPC = N_CTX // NCORES          # 512 queries per core
NKT = N_CTX // 128             # 32 key tiles
NDM = D_MODEL // 128           # 16 d_model chunks
MASK_NEG = -1.0e30

F16 = mybir.dt.float16
BF16 = mybir.dt.bfloat16
F32 = mybir.dt.float32


def _widths():
    # valid query-column width per key tile (tail slice [512-w : 512] of qT)
    return [QPC - 16 * kt for kt in range(NKT)]


def build_program():
    nc = bacc.Bacc(trn_type="TRN2", target_bir_lowering=False, debug=False)

    # ---- DRAM parameters (identical shapes on all cores; data differs) ----
    xqT = nc.declare_dram_parameter("xqT", [D_MODEL, QPC], F16, isOutput=False)
    xT = nc.declare_dram_parameter("xT", [D_MODEL, N_CTX], F16, isOutput=False)
    xv = nc.declare_dram_parameter("xv", [N_CTX, D_MODEL], BF16, isOutput=False)
    wqT = nc.declare_dram_parameter("wqT", [D_MODEL, D_HEAD], F16, isOutput=False)
    wkT = nc.declare_dram_parameter("wkT", [D_MODEL, D_HEAD], F16, isOutput=False)
    w2t = nc.declare_dram_parameter(
        "w2t", [NDM, NDM, 128, 128], F16, isOutput=False
    )  # w2t[ic, oc] = W2T[128ic:+128, 128oc:+128] (contiguous tiles)
    maskb = nc.declare_dram_parameter("maskb", [128, 16], F32, isOutput=False)
    outT = nc.declare_dram_parameter("outT", [D_MODEL, QPC], F32, isOutput=True)

    W = _widths()
    # packed column offsets of attnT tiles
    attn_off = np.concatenate([[0], np.cumsum(W)]).astype(int)
    attn_cols = int(attn_off[-1])  # 8448

    with tile.TileContext(nc) as tc:
        with (
            tc.tile_pool(name="static", bufs=1) as st,
            tc.tile_pool(name="xvpool", bufs=NKT) as xvp,
        ):
            # ---- resident tiles ----
            kT_sb = st.tile([128, N_CTX], F16, tag="kT")          # 1MB
            qT_sb = st.tile([128, QPC], F16, tag="qT")            # 0.13MB
            attn_sb = st.tile([128, attn_cols], BF16, tag="attn")  # 2.2MB
            ones_sb = st.tile([128, 1], BF16, tag="ones")
            mask_sb = st.tile([128, 16], F32, tag="mask")
            recip_sb = st.tile([128, QPC], F32, tag="recip")

            nc.vector.memset(ones_sb[:], 1.0)
            nc.sync.dma_start(out=mask_sb[:], in_=maskb[:])

            # xv resident: 32 tiles [128, 2048] bf16 (16.8MB), DMA'd early,
            # consumed tile-by-tile in phase 3.
            xv_t = []
            for kt in range(NKT):
                t = xvp.tile([128, D_MODEL], BF16, tag="xv")
                nc.sync.dma_start(out=t[:], in_=xv[128 * kt : 128 * (kt + 1), :])
                xv_t.append(t)

            # =========== Phase 1: projections qT, kT ===========
            with (
                tc.tile_pool(name="p1", bufs=1) as p1,
                tc.tile_pool(name="xts", bufs=8) as xts,
                tc.tile_pool(name="ps1", bufs=2, space="PSUM") as ps1,
            ):
                wq_t, wk_t, xq_t = [], [], []
                for ic in range(NDM):
                    tq = p1.tile([128, D_HEAD], F16, tag=f"wq{ic}")
                    nc.sync.dma_start(out=tq[:], in_=wqT[128 * ic : 128 * (ic + 1), :])
                    wq_t.append(tq)
                    tk = p1.tile([128, D_HEAD], F16, tag=f"wk{ic}")
                    nc.sync.dma_start(out=tk[:], in_=wkT[128 * ic : 128 * (ic + 1), :])
                    wk_t.append(tk)
                    tx = p1.tile([128, QPC], F16, tag=f"xq{ic}")
                    nc.sync.dma_start(out=tx[:], in_=xqT[128 * ic : 128 * (ic + 1), :])
                    xq_t.append(tx)

                # qT = WqT.T @ xqT  (accumulate over the 16 d_model chunks)
                psq = ps1.tile([128, QPC], F32, tag="psq")
                for ic in range(NDM):
                    nc.tensor.matmul(
                        psq[:], wq_t[ic][:], xq_t[ic][:],
                        start=(ic == 0), stop=(ic == NDM - 1),
                    )
                nc.vector.tensor_copy(qT_sb[:], psq[:])

                # kT in key groups of 512 columns
                for kg in range(N_CTX // 512):
                    psk = ps1.tile([128, 512], F32, tag="psk")
                    for ic in range(NDM):
                        tx = xts.tile([128, 512], F16, tag="xts")
                        nc.sync.dma_start(
                            out=tx[:],
                            in_=xT[128 * ic : 128 * (ic + 1), 512 * kg : 512 * (kg + 1)],
                        )
                        nc.tensor.matmul(
                            psk[:], wk_t[ic][:], tx[:],
                            start=(ic == 0), stop=(ic == NDM - 1),
                        )
                    nc.vector.tensor_copy(kT_sb[:, 512 * kg : 512 * (kg + 1)], psk[:])

            # =========== Phase 2: scores -> exp -> attnT (+ row sums) ======
            with (
                tc.tile_pool(name="ps2", bufs=2, space="PSUM") as ps2,
                tc.tile_pool(name="psd", bufs=1, space="PSUM") as psdp,
            ):
                psd = psdp.tile([1, QPC], F32, tag="psd")
                for kt in range(NKT):
                    w = W[kt]
                    o = int(attn_off[kt])
                    ps = ps2.tile([128, 512], F32, tag="pss")
                    # scoresT[kt] = kT[:,kt].T @ qT (contraction over d_head)
                    nc.tensor.matmul(
                        ps[:, :w],
                        kT_sb[:, 128 * kt : 128 * (kt + 1)],
                        qT_sb[:, QPC - w : QPC],
                        start=True, stop=True,
                    )
                    # additive causal mask on the 16 boundary columns
                    nc.vector.tensor_add(ps[:, :16], ps[:, :16], mask_sb[:])
                    # exp (unnormalized softmax numerator), cast to bf16
                    nc.scalar.activation(
                        attn_sb[:, o : o + w], ps[:, :w],
                        mybir.ActivationFunctionType.Exp,
                    )
                    # row sums: ones.T @ attnT accumulates into psd[0, tail]
                    nc.tensor.matmul(
                        psd[0:1, QPC - w : QPC],
                        ones_sb[:],
                        attn_sb[:, o : o + w],
                        start=(kt == 0), stop=(kt == NKT - 1),
                    )
                # reciprocal of row sums, broadcast to 128 partitions
                nc.vector.reciprocal(recip_sb[0:1, :], psd[0:1, :])
                nc.gpsimd.partition_broadcast(recip_sb[:], recip_sb[0:1, :])

            # =========== Phase 3: attn_outT = x.T @ attnT ===========
            with tc.tile_pool(name="p34", bufs=1) as p34:
                ao_t = []
                with tc.tile_pool(name="ps3", bufs=8, space="PSUM") as ps3:
                    for h in range(2):
                        pso = [ps3.tile([128, QPC], F32, tag="pso", name=f"pso{h}_{j}") for j in range(8)]
                        for kt in range(NKT):
                            w = W[kt]
                            o = int(attn_off[kt])
                            for j in range(8):
                                oc = 8 * h + j
                                nc.tensor.matmul(
                                    pso[j][:, QPC - w : QPC],
                                    xv_t[kt][:, 128 * oc : 128 * (oc + 1)],
                                    attn_sb[:, o : o + w],
                                    start=(kt == 0), stop=(kt == NKT - 1),
                                )
                        for j in range(8):
                            oc = 8 * h + j
                            t = p34.tile([128, QPC], F16, tag=f"ao{oc}")
                            # normalize by softmax row-sums here so fp16 stays in range
                            nc.vector.tensor_mul(t[:], pso[j][:], recip_sb[:])
                            ao_t.append(t)

                # =========== Phase 4: outT = W2T.T @ attn_outT ==
                with (
                    tc.tile_pool(name="w2s", bufs=16, space="SBUF") as w2s,
                    tc.tile_pool(name="outs", bufs=4) as outs,
                    tc.tile_pool(name="ps4", bufs=2, space="PSUM") as ps4,
                ):
                    for oc in range(NDM):
                        ps = ps4.tile([128, QPC], F32, tag="ps4")
                        for ic in range(NDM):
                            tw = w2s.tile([128, 128], F16, tag="w2")
                            nc.sync.dma_start(out=tw[:], in_=w2t[ic, oc])
                            nc.tensor.matmul(
                                ps[:], tw[:], ao_t[ic][:],
                                start=(ic == 0), stop=(ic == NDM - 1),
                            )
                        t = outs.tile([128, QPC], F32, tag="out")
                        nc.vector.tensor_copy(t[:], ps[:])
                        nc.sync.dma_start(
                            out=outT[128 * oc : 128 * (oc + 1), :], in_=t[:]
                        )

    nc.compile()
    return nc


def prepare_inputs(x, Wk, Wq, W2):
    """Host-side sharding/layout prep. Returns in_maps for the 8 cores."""
    x = np.asarray(x, dtype=np.float32)
    Wk = np.asarray(Wk, dtype=np.float32)
    Wq = np.asarray(Wq, dtype=np.float32)
    W2 = np.asarray(W2, dtype=np.float32)

    xT16 = np.ascontiguousarray(x.T).astype(np.float16)          # [D, N]
    xv16 = x.astype(ml_dtypes.bfloat16)                          # [N, D]
    wqT = np.ascontiguousarray(Wq.T).astype(np.float16)          # [D, H]
    wkT = np.ascontiguousarray(Wk.T).astype(np.float16)
    w2T = np.ascontiguousarray(W2.T).astype(np.float16)          # [D_in, D_out]
    w2tiles = np.ascontiguousarray(
        w2T.reshape(NDM, 128, NDM, 128).transpose(0, 2, 1, 3)
    )  # [ic, oc, 128, 128]

    in_maps = []
    for c in range(NCORES):
        # queries 8m+c  ->  xqT = x[c::8].T
        xqT = np.ascontiguousarray(x[c::NCORES].T).astype(np.float16)
        mask = np.zeros((128, 16), dtype=np.float32)
        j = np.arange(128)[:, None]
        t = np.arange(16)[None, :]
        mask[j > 8 * t + c] = MASK_NEG
        in_maps.append(
            {
                "xqT": xqT,
                "xT": xT16,
                "xv": xv16,
                "wqT": wqT,
                "wkT": wkT,
                "w2t": w2tiles,
                "maskb": mask,
            }
        )
    return in_maps


def assemble_output(results):
    # results[c]["outT"] is [D_MODEL, QPC] f32 for queries 8m+c
    res = np.stack([np.asarray(results[c]["outT"]) for c in range(NCORES)])
    # [c, d, m] -> out[8m+c, d]
    return np.ascontiguousarray(res.transpose(2, 0, 1).reshape(N_CTX, D_MODEL))


_CACHED = {}


def kernel(x, Wk, Wq, W2, _trace=False):
    if "nc" not in _CACHED:
        _CACHED["nc"] = build_program()
    nc = _CACHED["nc"]
    in_maps = prepare_inputs(x, Wk, Wq, W2)
    res = run_bass_kernel_spmd(nc, in_maps, core_ids=list(range(NCORES)), trace=_trace)
    out = assemble_output(res.results)
    if _trace:
        return out, res
    return out


# revision 9
# speedup vs baseline: 1.6229x; 1.6229x over previous
"""Causal attention kernel for TRN2, 8 NeuronCores (SPMD).

Problem:  x[4096,2048] f32; q = x@Wq.T, k = x@Wk.T (d_head=128),
          scores = q@k.T causal-masked, attn = softmax(scores),
          out = (attn @ x) @ W2.T.

Sharding: sequence-parallel over queries with stride-8 interleave:
  core c owns queries {8m+c : m=0..511}.  For key tile kt (128 keys),
  every core has exactly 512-16*kt valid queries -- a contiguous tail
  slice of its query columns -- so the SPMD program is identical on all
  cores (no dynamic control flow, no collectives) and causal work is
  perfectly balanced.

Precision: fp16 inputs for the q/k projections and the score matmul
  (fp32 PSUM accumulation), unnormalized softmax (exp without
  max-subtraction: scores are bounded ~|s|<70 for unit-normal inputs,
  safely inside fp32 exp range), attention weights in bf16 (bf16 has
  fp32 exponent range, needed for exp(s) up to ~1e28), V and W2
  matmuls in bf16/fp16, normalization by the softmax row-sum applied
  at the attn_out eviction (keeps fp16 in range).

Layouts are all "transposed" (d_head / d_model on the partition dim) so
no on-device transposes are needed; the host pre-transposes inputs and
inverse-permutes the gathered outputs.  Weight/activation inputs are
host-packed so each consumer tile group loads with ONE big DMA (HWDGE
issue cost is ~0.6us per dma_start; v1 with 481 issues was
issue-bound).  xv (V-matmul operand) loads go through the scalar
engine's HWDGE queue to keep the sync queue free for the critical path.
"""

import numpy as np
import ml_dtypes

import concourse.bass as bass
import concourse.bacc as bacc
import concourse.mybir as mybir
import concourse.tile as tile
from concourse.bass_utils import run_bass_kernel_spmd

N_CTX = 4096
D_MODEL = 2048
D_HEAD = 128
NCORES = 8
QPC = N_CTX // NCORES          # 512 queries per core
NKT = N_CTX // 128             # 32 key tiles
NDM = D_MODEL // 128           # 16 d_model chunks
MASK_NEG = -1.0e30

F16 = mybir.dt.float16
BF16 = mybir.dt.bfloat16
F32 = mybir.dt.float32


def _widths():
    # valid query-column width per key tile (tail slice [512-w : 512] of qT)
    return [QPC - 16 * kt for kt in range(NKT)]


def build_program():
    nc = bacc.Bacc(trn_type="TRN2", target_bir_lowering=False, debug=False)

    # ---- DRAM parameters (identical shapes on all cores; data differs) ----
    # xqr[r, 512*ic + m] = x[8m+c, 128*ic + r]   (own-query columns, packed)
    xqr = nc.declare_dram_parameter("xqr", [128, NDM * QPC], F16, isOutput=False)
    # xT[d, n] = x[n, d]
    xT = nc.declare_dram_parameter("xT", [D_MODEL, N_CTX], F16, isOutput=False)
    # xv = x (natural layout), bf16
    xv = nc.declare_dram_parameter("xv", [N_CTX, D_MODEL], BF16, isOutput=False)
    # wqr[r, 128*ic + h] = Wq[h, 128*ic + r]; same for wkr
    wqr = nc.declare_dram_parameter("wqr", [128, D_MODEL], F16, isOutput=False)
    wkr = nc.declare_dram_parameter("wkr", [128, D_MODEL], F16, isOutput=False)
    # w2r[oc][r, 128*ic + o] = W2[128*oc + o, 128*ic + r]
    w2r = nc.declare_dram_parameter("w2r", [NDM, 128, D_MODEL], F16, isOutput=False)
    maskb = nc.declare_dram_parameter("maskb", [128, 16], F32, isOutput=False)
    outT = nc.declare_dram_parameter("outT", [D_MODEL, QPC], F32, isOutput=True)

    W = _widths()
    attn_off = np.concatenate([[0], np.cumsum(W)]).astype(int)
    attn_cols = int(attn_off[-1])  # 8448

    with tile.TileContext(nc) as tc:
        with (
            tc.tile_pool(name="static", bufs=1) as st,
            tc.tile_pool(name="xvpool", bufs=NKT) as xvp,
        ):
            # ---- resident tiles ----
            kT_sb = st.tile([128, N_CTX], F16, tag="kT")
            qT_sb = st.tile([128, QPC], F16, tag="qT")
            attn_sb = st.tile([128, attn_cols], BF16, tag="attn")
            ones_sb = st.tile([128, 1], BF16, tag="ones")
            mask_sb = st.tile([128, 16], F32, tag="mask")
            recip_sb = st.tile([128, QPC], F32, tag="recip")

            nc.vector.memset(ones_sb[:], 1.0)
            nc.sync.dma_start(out=mask_sb[:], in_=maskb[:])

            # xv resident: 32 tiles [128, 2048] bf16 via the scalar HWDGE queue
            xv_t = []
            for kt in range(NKT):
                t = xvp.tile([128, D_MODEL], BF16, tag="xv")
                nc.scalar.dma_start(out=t[:], in_=xv[128 * kt : 128 * (kt + 1), :])
                xv_t.append(t)

            # =========== Phase 1: projections qT, kT ===========
            with (
                tc.tile_pool(name="p1", bufs=1) as p1,
                tc.tile_pool(name="xts", bufs=3) as xts,
            ):
                wq_sb = p1.tile([128, D_MODEL], F16, tag="wq")
                nc.sync.dma_start(out=wq_sb[:], in_=wqr[:])
                wk_sb = p1.tile([128, D_MODEL], F16, tag="wk")
                nc.sync.dma_start(out=wk_sb[:], in_=wkr[:])
                xq_sb = p1.tile([128, NDM * QPC], F16, tag="xq")
                nc.sync.dma_start(out=xq_sb[:], in_=xqr[:])

                with tc.tile_pool(name="ps1q", bufs=1, space="PSUM") as ps1q:
                    psq = ps1q.tile([128, QPC], F32, tag="psq")
                    for ic in range(NDM):
                        nc.tensor.matmul(
                            psq[:],
                            wq_sb[:, 128 * ic : 128 * (ic + 1)],
                            xq_sb[:, QPC * ic : QPC * (ic + 1)],
                            start=(ic == 0), stop=(ic == NDM - 1),
                        )
                    nc.vector.tensor_copy(qT_sb[:], psq[:])

                # kT in key groups of 256 columns; one 3D-strided DMA per group
                KG = 256
                xT3 = xT.rearrange("(i r) n -> r i n", r=128)  # [128, 16, 4096]
                with tc.tile_pool(name="ps1k", bufs=2, space="PSUM") as ps1k:
                    for kg in range(N_CTX // KG):
                        t = xts.tile([128, NDM * KG], F16, tag="xts")
                        nc.sync.dma_start(
                            out=t[:].rearrange("r (i n) -> r i n", n=KG),
                            in_=xT3[:, :, KG * kg : KG * (kg + 1)],
                        )
                        psk = ps1k.tile([128, KG], F32, tag="psk")
                        for ic in range(NDM):
                            nc.tensor.matmul(
                                psk[:],
                                wk_sb[:, 128 * ic : 128 * (ic + 1)],
                                t[:, KG * ic : KG * (ic + 1)],
                                start=(ic == 0), stop=(ic == NDM - 1),
                            )
                        nc.vector.tensor_copy(
                            kT_sb[:, KG * kg : KG * (kg + 1)], psk[:]
                        )

            # =========== Phase 2: scores -> exp -> attnT (+ row sums) ======
            with (
                tc.tile_pool(name="ps2", bufs=2, space="PSUM") as ps2,
                tc.tile_pool(name="psd", bufs=1, space="PSUM") as psdp,
            ):
                psd = psdp.tile([1, QPC], F32, tag="psd")
                for kt in range(NKT):
                    w = W[kt]
                    o = int(attn_off[kt])
                    ps = ps2.tile([128, 512], F32, tag="pss")
                    nc.tensor.matmul(
                        ps[:, :w],
                        kT_sb[:, 128 * kt : 128 * (kt + 1)],
                        qT_sb[:, QPC - w : QPC],
                        start=True, stop=True,
                    )
                    # additive causal mask on the 16 boundary columns
                    nc.vector.tensor_add(ps[:, :16], ps[:, :16], mask_sb[:])
                    nc.scalar.activation(
                        attn_sb[:, o : o + w], ps[:, :w],
                        mybir.ActivationFunctionType.Exp,
                    )
                    nc.tensor.matmul(
                        psd[0:1, QPC - w : QPC],
                        ones_sb[:],
                        attn_sb[:, o : o + w],
                        start=(kt == 0), stop=(kt == NKT - 1),
                    )
                nc.vector.reciprocal(recip_sb[0:1, :], psd[0:1, :])
                nc.gpsimd.partition_broadcast(recip_sb[:], recip_sb[0:1, :])

            # =========== Phase 3: attn_outT = x.T @ attnT ===========
            with tc.tile_pool(name="p34", bufs=1) as p34:
                ao_t = []
                with tc.tile_pool(name="ps3", bufs=8, space="PSUM") as ps3:
                    for h in range(2):
                        pso = [
                            ps3.tile([128, QPC], F32, tag="pso", name=f"pso{h}_{j}")
                            for j in range(8)
                        ]
                        for kt in range(NKT):
                            w = W[kt]
                            o = int(attn_off[kt])
                            for j in range(8):
                                oc = 8 * h + j
                                nc.tensor.matmul(
                                    pso[j][:, QPC - w : QPC],
                                    xv_t[kt][:, 128 * oc : 128 * (oc + 1)],
                                    attn_sb[:, o : o + w],
                                    start=(kt == 0), stop=(kt == NKT - 1),
                                )
                        for j in range(8):
                            oc = 8 * h + j
                            t = p34.tile([128, QPC], F16, tag=f"ao{oc}")
                            # normalize here so fp16 stays in range
                            nc.vector.tensor_mul(t[:], pso[j][:], recip_sb[:])
                            ao_t.append(t)

                # =========== Phase 4: outT = W2T.T @ attn_outT ==
                with (
                    tc.tile_pool(name="w2s", bufs=4, space="SBUF") as w2s,
                    tc.tile_pool(name="outs", bufs=4) as outs,
                    tc.tile_pool(name="ps4", bufs=2, space="PSUM") as ps4,
                ):
                    for oc in range(NDM):
                        tw = w2s.tile([128, D_MODEL], F16, tag="w2")
                        nc.sync.dma_start(out=tw[:], in_=w2r[oc])
                        ps = ps4.tile([128, QPC], F32, tag="ps4")
                        for ic in range(NDM):
                            nc.tensor.matmul(
                                ps[:],
                                tw[:, 128 * ic : 128 * (ic + 1)],
                                ao_t[ic][:],
                                start=(ic == 0), stop=(ic == NDM - 1),
                            )
                        t = outs.tile([128, QPC], F32, tag="out")
                        nc.vector.tensor_copy(t[:], ps[:])
                        nc.sync.dma_start(
                            out=outT[128 * oc : 128 * (oc + 1), :], in_=t[:]
                        )

    nc.compile()
    return nc


def prepare_inputs(x, Wk, Wq, W2):
    """Host-side sharding/layout prep. Returns in_maps for the 8 cores."""
    x = np.asarray(x, dtype=np.float32)
    Wk = np.asarray(Wk, dtype=np.float32)
    Wq = np.asarray(Wq, dtype=np.float32)
    W2 = np.asarray(W2, dtype=np.float32)

    xT16 = np.ascontiguousarray(x.T).astype(np.float16)          # [D, N]
    xv16 = x.astype(ml_dtypes.bfloat16)                          # [N, D]

    def pack_chunks(aT, width):
        # aT [D_MODEL, width] -> [128, NDM*width]: out[r, width*ic + c] = aT[128ic+r, c]
        return np.ascontiguousarray(
            aT.reshape(NDM, 128, width).transpose(1, 0, 2).reshape(128, NDM * width)
        )

    wqr = pack_chunks(np.ascontiguousarray(Wq.T).astype(np.float16), D_HEAD)
    wkr = pack_chunks(np.ascontiguousarray(Wk.T).astype(np.float16), D_HEAD)
    # w2r[oc, r, 128*ic + o] = W2T[128ic+r, 128oc+o]
    w2T = np.ascontiguousarray(W2.T).astype(np.float16)
    w2r = np.ascontiguousarray(
        w2T.reshape(NDM, 128, NDM, 128).transpose(2, 1, 0, 3).reshape(NDM, 128, D_MODEL)
    )

    in_maps = []
    for c in range(NCORES):
        xqT = np.ascontiguousarray(x[c::NCORES].T).astype(np.float16)  # [D, QPC]
        xqr = pack_chunks(xqT, QPC)
        mask = np.zeros((128, 16), dtype=np.float32)
        j = np.arange(128)[:, None]
        t = np.arange(16)[None, :]
        mask[j > 8 * t + c] = MASK_NEG
        in_maps.append(
            {
                "xqr": xqr,
                "xT": xT16,
                "xv": xv16,
                "wqr": wqr,
                "wkr": wkr,
                "w2r": w2r,
                "maskb": mask,
            }
        )
    return in_maps


def assemble_output(results):
    res = np.stack([np.asarray(results[c]["outT"]) for c in range(NCORES)])
    # [c, d, m] -> out[8m+c, d]
    return np.ascontiguousarray(res.transpose(2, 0, 1).reshape(N_CTX, D_MODEL))


_CACHED = {}


def kernel(x, Wk, Wq, W2, _trace=False):
    if "nc" not in _CACHED:
        _CACHED["nc"] = build_program()
    nc = _CACHED["nc"]
    in_maps = prepare_inputs(x, Wk, Wq, W2)
    res = run_bass_kernel_spmd(nc, in_maps, core_ids=list(range(NCORES)), trace=_trace)
    out = assemble_output(res.results)
    if _trace:
        return out, res
    return out


# revision 10
# speedup vs baseline: 1.6477x; 1.0153x over previous
"""Causal attention kernel for TRN2, 8 NeuronCores (SPMD).

Problem:  x[4096,2048] f32; q = x@Wq.T, k = x@Wk.T (d_head=128),
          scores = q@k.T causal-masked, attn = softmax(scores),
          out = (attn @ x) @ W2.T.

Sharding: sequence-parallel over queries with stride-8 interleave:
  core c owns queries {8m+c : m=0..511}.  For key tile kt (128 keys),
  every core has exactly 512-16*kt valid queries -- a contiguous tail
  slice of its query columns -- so the SPMD program is identical on all
  cores (no dynamic control flow, no collectives) and causal work is
  perfectly balanced.

Precision: fp16 inputs for the q/k projections and the score matmul
  (fp32 PSUM accumulation), unnormalized softmax (exp without
  max-subtraction: scores are bounded ~|s|<70 for unit-normal inputs,
  safely inside fp32 exp range), attention weights in bf16 (bf16 has
  fp32 exponent range, needed for exp(s) up to ~1e28), V and W2
  matmuls in bf16/fp16, normalization by the softmax row-sum applied
  at the attn_out eviction (keeps fp16 in range).

Layouts are all "transposed" (d_head / d_model on the partition dim) so
no on-device transposes are needed; the host pre-transposes inputs and
inverse-permutes the gathered outputs.  Weight/activation inputs are
host-packed so each consumer tile group loads with ONE big DMA (HWDGE
issue cost is ~0.6us per dma_start; v1 with 481 issues was
issue-bound).  xv (V-matmul operand) loads go through the scalar
engine's HWDGE queue to keep the sync queue free for the critical path.
"""

import numpy as np
import ml_dtypes

import concourse.bass as bass
import concourse.bacc as bacc
import concourse.mybir as mybir
import concourse.tile as tile
from concourse.bass_utils import run_bass_kernel_spmd

N_CTX = 4096
D_MODEL = 2048
D_HEAD = 128
NCORES = 8
QPC = N_CTX // NCORES          # 512 queries per core
NKT = N_CTX // 128             # 32 key tiles
NDM = D_MODEL // 128           # 16 d_model chunks
MASK_NEG = -1.0e30

F16 = mybir.dt.float16
BF16 = mybir.dt.bfloat16
F32 = mybir.dt.float32


def _widths():
    # valid query-column width per key tile (tail slice [512-w : 512] of qT)
    return [QPC - 16 * kt for kt in range(NKT)]


def build_program():
    nc = bacc.Bacc(trn_type="TRN2", target_bir_lowering=False, debug=False)

    # ---- DRAM parameters (identical shapes on all cores; data differs) ----
    # xqr[r, 512*ic + m] = x[8m+c, 128*ic + r]   (own-query columns, packed)
    xqr = nc.declare_dram_parameter("xqr", [128, NDM * QPC], F16, isOutput=False)
    # xkr[r, 512*ic + n] = x[512*c + n, 128*ic + r]  (own key-shard cols, packed)
    xkr = nc.declare_dram_parameter("xkr", [128, NDM * 512], F16, isOutput=False)
    # xv = x (natural layout), bf16
    xv = nc.declare_dram_parameter("xv", [N_CTX, D_MODEL], BF16, isOutput=False)
    # wqr[r, 128*ic + h] = Wq[h, 128*ic + r]; same for wkr
    wqr = nc.declare_dram_parameter("wqr", [128, D_MODEL], F16, isOutput=False)
    wkr = nc.declare_dram_parameter("wkr", [128, D_MODEL], F16, isOutput=False)
    # w2r[oc][r, 128*ic + o] = W2[128*oc + o, 128*ic + r]
    w2r = nc.declare_dram_parameter("w2r", [NDM, 128, D_MODEL], F16, isOutput=False)
    maskb = nc.declare_dram_parameter("maskb", [128, 16], F32, isOutput=False)
    outT = nc.declare_dram_parameter("outT", [D_MODEL, QPC], F32, isOutput=True)

    W = _widths()
    attn_off = np.concatenate([[0], np.cumsum(W)]).astype(int)
    attn_cols = int(attn_off[-1])  # 8448

    with tile.TileContext(nc) as tc:
        with (
            tc.tile_pool(name="static", bufs=1) as st,
            tc.tile_pool(name="xvpool", bufs=NKT) as xvp,
        ):
            # ---- resident tiles ----
            kT_sb = st.tile([128, N_CTX], F16, tag="kT")
            qT_sb = st.tile([128, QPC], F16, tag="qT")
            attn_sb = st.tile([128, attn_cols], BF16, tag="attn")
            ones_sb = st.tile([128, 1], BF16, tag="ones")
            mask_sb = st.tile([128, 16], F32, tag="mask")
            recip_sb = st.tile([128, QPC], F32, tag="recip")

            nc.vector.memset(ones_sb[:], 1.0)
            nc.sync.dma_start(out=mask_sb[:], in_=maskb[:])

            # =========== Phase 1: projections qT + sharded kT, AllGather ===
            with (
                tc.tile_pool(name="p1", bufs=1) as p1,
                tc.tile_pool(name="dramp", bufs=1, space="DRAM") as dramp,
            ):
                wq_sb = p1.tile([128, D_MODEL], F16, tag="wq")
                nc.sync.dma_start(out=wq_sb[:], in_=wqr[:])
                wk_sb = p1.tile([128, D_MODEL], F16, tag="wk")
                nc.sync.dma_start(out=wk_sb[:], in_=wkr[:])
                xq_sb = p1.tile([128, NDM * QPC], F16, tag="xq")
                nc.sync.dma_start(out=xq_sb[:], in_=xqr[:])
                xk_sb = p1.tile([128, NDM * 512], F16, tag="xk")
                nc.sync.dma_start(out=xk_sb[:], in_=xkr[:])

                # kT shard first (it gates the AllGather -> scores)
                kt_shard = p1.tile([128, 512], F16, tag="kts")
                with tc.tile_pool(name="ps1k", bufs=1, space="PSUM") as ps1k:
                    psk = ps1k.tile([128, 512], F32, tag="psk")
                    for ic in range(NDM):
                        nc.tensor.matmul(
                            psk[:],
                            wk_sb[:, 128 * ic : 128 * (ic + 1)],
                            xk_sb[:, 512 * ic : 512 * (ic + 1)],
                            start=(ic == 0), stop=(ic == NDM - 1),
                        )
                    nc.vector.tensor_copy(kt_shard[:], psk[:])

                kt_in = dramp.tile([128, 512], F16, tag="ktin")
                kt_all = dramp.tile([NCORES * 128, 512], F16, tag="ktall",
                                    addr_space="Shared")
                nc.gpsimd.dma_start(out=kt_in[:], in_=kt_shard[:])
                nc.gpsimd.collective_compute(
                    "AllGather",
                    mybir.AluOpType.bypass,
                    replica_groups=[list(range(NCORES))],
                    ins=[kt_in[:].opt()],
                    outs=[kt_all[:].opt()],
                )
                # kT_sb[:, 512*s + n] = kt_all[128*s + r, n]
                nc.sync.dma_start(
                    out=kT_sb[:].rearrange("r (s n) -> r s n", n=512),
                    in_=kt_all[:].rearrange("(s r) n -> r s n", r=128),
                )

                with tc.tile_pool(name="ps1q", bufs=1, space="PSUM") as ps1q:
                    psq = ps1q.tile([128, QPC], F32, tag="psq")
                    for ic in range(NDM):
                        nc.tensor.matmul(
                            psq[:],
                            wq_sb[:, 128 * ic : 128 * (ic + 1)],
                            xq_sb[:, QPC * ic : QPC * (ic + 1)],
                            start=(ic == 0), stop=(ic == NDM - 1),
                        )
                    nc.vector.tensor_copy(qT_sb[:], psq[:])

            # xv resident: 32 tiles [128, 2048] bf16 via the scalar HWDGE queue
            # (emitted after phase 1 so the early DMA window is free for the
            # projection inputs that gate the score pipeline)
            xv_t = []
            for kt in range(NKT):
                t = xvp.tile([128, D_MODEL], BF16, tag="xv", name=f"xv{kt}")
                nc.scalar.dma_start(out=t[:], in_=xv[128 * kt : 128 * (kt + 1), :])
                xv_t.append(t)

            # =========== Phase 2: scores -> exp -> attnT (+ row sums) ======
            with (
                tc.tile_pool(name="ps2", bufs=3, space="PSUM") as ps2,
                tc.tile_pool(name="psd", bufs=1, space="PSUM") as psdp,
            ):
                psd = psdp.tile([1, QPC], F32, tag="psd")
                for kt in range(NKT):
                    w = W[kt]
                    o = int(attn_off[kt])
                    ps = ps2.tile([128, 512], F32, tag="pss")
                    nc.tensor.matmul(
                        ps[:, :w],
                        kT_sb[:, 128 * kt : 128 * (kt + 1)],
                        qT_sb[:, QPC - w : QPC],
                        start=True, stop=True,
                    )
                    # additive causal mask on the 16 boundary columns
                    nc.vector.tensor_add(ps[:, :16], ps[:, :16], mask_sb[:])
                    nc.scalar.activation(
                        attn_sb[:, o : o + w], ps[:, :w],
                        mybir.ActivationFunctionType.Exp,
                    )
                    nc.tensor.matmul(
                        psd[0:1, QPC - w : QPC],
                        ones_sb[:],
                        attn_sb[:, o : o + w],
                        start=(kt == 0), stop=(kt == NKT - 1),
                    )
                nc.vector.reciprocal(recip_sb[0:1, :], psd[0:1, :])
                nc.gpsimd.partition_broadcast(recip_sb[:], recip_sb[0:1, :])

            # =========== Phase 3: attn_outT = x.T @ attnT ===========
            with tc.tile_pool(name="p34", bufs=1) as p34:
                ao_t = []
                with tc.tile_pool(name="ps3", bufs=8, space="PSUM") as ps3:
                    for h in range(2):
                        pso = [
                            ps3.tile([128, QPC], F32, tag="pso", name=f"pso{h}_{j}")
                            for j in range(8)
                        ]
                        for kt in range(NKT):
                            w = W[kt]
                            o = int(attn_off[kt])
                            for j in range(8):
                                oc = 8 * h + j
                                nc.tensor.matmul(
                                    pso[j][:, QPC - w : QPC],
                                    xv_t[kt][:, 128 * oc : 128 * (oc + 1)],
                                    attn_sb[:, o : o + w],
                                    start=(kt == 0), stop=(kt == NKT - 1),
                                )
                        for j in range(8):
                            oc = 8 * h + j
                            t = p34.tile([128, QPC], F16, tag=f"ao{oc}")
                            # normalize here so fp16 stays in range
                            nc.vector.tensor_mul(t[:], pso[j][:], recip_sb[:])
                            ao_t.append(t)

                # =========== Phase 4: outT = W2T.T @ attn_outT ==
                with (
                    tc.tile_pool(name="w2s", bufs=4, space="SBUF") as w2s,
                    tc.tile_pool(name="outs", bufs=4) as outs,
                    tc.tile_pool(name="ps4", bufs=2, space="PSUM") as ps4,
                ):
                    for oc in range(NDM):
                        tw = w2s.tile([128, D_MODEL], F16, tag="w2")
                        nc.sync.dma_start(out=tw[:], in_=w2r[oc])
                        ps = ps4.tile([128, QPC], F32, tag="ps4")
                        for ic in range(NDM):
                            nc.tensor.matmul(
                                ps[:],
                                tw[:, 128 * ic : 128 * (ic + 1)],
                                ao_t[ic][:],
                                start=(ic == 0), stop=(ic == NDM - 1),
                            )
                        t = outs.tile([128, QPC], F32, tag="out")
                        nc.vector.tensor_copy(t[:], ps[:])
                        nc.sync.dma_start(
                            out=outT[128 * oc : 128 * (oc + 1), :], in_=t[:]
                        )

    nc.compile()
    return nc


def prepare_inputs(x, Wk, Wq, W2):
    """Host-side sharding/layout prep. Returns in_maps for the 8 cores."""
    x = np.asarray(x, dtype=np.float32)
    Wk = np.asarray(Wk, dtype=np.float32)
    Wq = np.asarray(Wq, dtype=np.float32)
    W2 = np.asarray(W2, dtype=np.float32)

    xv16 = x.astype(ml_dtypes.bfloat16)                          # [N, D]

    def pack_chunks(aT, width):
        # aT [D_MODEL, width] -> [128, NDM*width]: out[r, width*ic + c] = aT[128ic+r, c]
        return np.ascontiguousarray(
            aT.reshape(NDM, 128, width).transpose(1, 0, 2).reshape(128, NDM * width)
        )

    wqr = pack_chunks(np.ascontiguousarray(Wq.T).astype(np.float16), D_HEAD)
    wkr = pack_chunks(np.ascontiguousarray(Wk.T).astype(np.float16), D_HEAD)
    # w2r[oc, r, 128*ic + o] = W2T[128ic+r, 128oc+o]
    w2T = np.ascontiguousarray(W2.T).astype(np.float16)
    w2r = np.ascontiguousarray(
        w2T.reshape(NDM, 128, NDM, 128).transpose(2, 1, 0, 3).reshape(NDM, 128, D_MODEL)
    )

    in_maps = []
    for c in range(NCORES):
        xqT = np.ascontiguousarray(x[c::NCORES].T).astype(np.float16)  # [D, QPC]
        xqr = pack_chunks(xqT, QPC)
        xkT = np.ascontiguousarray(x[512 * c : 512 * (c + 1)].T).astype(np.float16)
        xkr_c = pack_chunks(xkT, 512)
        mask = np.zeros((128, 16), dtype=np.float32)
        j = np.arange(128)[:, None]
        t = np.arange(16)[None, :]
        mask[j > 8 * t + c] = MASK_NEG
        in_maps.append(
            {
                "xqr": xqr,
                "xkr": xkr_c,
                "xv": xv16,
                "wqr": wqr,
                "wkr": wkr,
                "w2r": w2r,
                "maskb": mask,
            }
        )
    return in_maps


def assemble_output(results):
    res = np.stack([np.asarray(results[c]["outT"]) for c in range(NCORES)])
    # [c, d, m] -> out[8m+c, d]
    return np.ascontiguousarray(res.transpose(2, 0, 1).reshape(N_CTX, D_MODEL))


_CACHED = {}


def kernel(x, Wk, Wq, W2, _trace=False):
    if "nc" not in _CACHED:
        _CACHED["nc"] = build_program()
    nc = _CACHED["nc"]
    in_maps = prepare_inputs(x, Wk, Wq, W2)
    res = run_bass_kernel_spmd(nc, in_maps, core_ids=list(range(NCORES)), trace=_trace)
    out = assemble_output(res.results)
    if _trace:
        return out, res
    return out


# revision 13
# speedup vs baseline: 1.9487x; 1.1827x over previous
"""Causal attention kernel for TRN2, 8 NeuronCores (SPMD).

Problem:  x[4096,2048] f32; q = x@Wq.T, k = x@Wk.T (d_head=128),
          scores = q@k.T causal-masked, attn = softmax(scores),
          out = (attn @ x) @ W2.T.

Sharding: sequence-parallel over queries with stride-8 interleave:
  core c owns queries {8m+c : m=0..511}.  For key tile kt (128 keys),
  every core has exactly 512-16*kt valid queries -- a contiguous tail
  slice of its query columns -- so the SPMD program is identical on all
  cores (no dynamic control flow, no collectives) and causal work is
  perfectly balanced.

Precision: fp16 inputs for the q/k projections and the score matmul
  (fp32 PSUM accumulation), unnormalized softmax (exp without
  max-subtraction: scores are bounded ~|s|<70 for unit-normal inputs,
  safely inside fp32 exp range), attention weights in bf16 (bf16 has
  fp32 exponent range, needed for exp(s) up to ~1e28), V and W2
  matmuls in bf16/fp16, normalization by the softmax row-sum applied
  at the attn_out eviction (keeps fp16 in range).

Scheduling notes (v4):
  * ALL loads go through the sync engine's HWDGE queues in priority
    order (small projection inputs -> xT stream -> xv -> W2).  The 8
    queues are drained round-robin with per-queue FIFO, so issue order
    == byte order; nothing starves the critical path (v2/v3 lost
    ~30us to xv flooding a parallel queue set at t=0).
  * kT / attnT are split into per-keytile tiles so Tile's dependency
    tracking lets scores/exp/V-matmuls pipeline INTO the xT DMA
    stream instead of waiting for a whole-tensor barrier.
  * The V matmul for output chunks 0-3 is fused into the score loop
    (PSUM budget: kT 1 + scores 2 + denom 1 + V 4 = 8 banks); chunks
    4-15 run right after from SBUF-resident attnT.
  * An AllGather-based sharded-kT variant was measured: the 8-core
    0.5MB AllGather costs ~100us on this runtime -- slower than just
    replicating the kT projection (27us compute, overlapped DMA).
"""

from contextlib import ExitStack

import numpy as np
import ml_dtypes

import concourse.bass as bass
import concourse.bacc as bacc
import concourse.mybir as mybir
import concourse.tile as tile
from concourse.bass_utils import run_bass_kernel_spmd

N_CTX = 4096
D_MODEL = 2048
D_HEAD = 128
NCORES = 8
QPC = N_CTX // NCORES          # 512 queries per core
NKT = N_CTX // 128             # 32 key tiles
NDM = D_MODEL // 128           # 16 d_model chunks
KG = 256                       # kT projection key-group width
NKG = N_CTX // KG
MASK_NEG = -1.0e30

F16 = mybir.dt.float16
BF16 = mybir.dt.bfloat16
F32 = mybir.dt.float32


def _widths():
    # valid query-column width per key tile (tail slice [512-w : 512] of qT)
    return [QPC - 16 * kt for kt in range(NKT)]


def build_program():
    nc = bacc.Bacc(trn_type="TRN2", target_bir_lowering=False, debug=False)

    # ---- DRAM parameters (identical shapes on all cores; data differs) ----
    # xqr[r, 512*ic + m] = x[8m+c, 128*ic + r]   (own-query columns, packed)
    xqr = nc.declare_dram_parameter("xqr", [128, NDM * QPC], F16, isOutput=False)
    # xT[d, n] = x[n, d]
    xT = nc.declare_dram_parameter("xT", [D_MODEL, N_CTX], F16, isOutput=False)
    # xv = x (natural layout), bf16
    xv = nc.declare_dram_parameter("xv", [N_CTX, D_MODEL], BF16, isOutput=False)
    # wqr[r, 128*ic + h] = Wq[h, 128*ic + r]; same for wkr
    wqr = nc.declare_dram_parameter("wqr", [128, D_MODEL], F16, isOutput=False)
    wkr = nc.declare_dram_parameter("wkr", [128, D_MODEL], F16, isOutput=False)
    # w2r[oc][r, 128*ic + o] = W2[128*oc + o, 128*ic + r]
    w2r = nc.declare_dram_parameter("w2r", [NDM, 128, D_MODEL], F16, isOutput=False)
    maskb = nc.declare_dram_parameter("maskb", [128, 16], F32, isOutput=False)
    outT = nc.declare_dram_parameter("outT", [D_MODEL, QPC], F32, isOutput=True)

    W = _widths()

    with tile.TileContext(nc) as tc:
        with (
            tc.tile_pool(name="static", bufs=1) as st,
            tc.tile_pool(name="xvpool", bufs=NKT) as xvp,
            tc.tile_pool(name="ktpool", bufs=NKG) as ktp,
            tc.tile_pool(name="atpool", bufs=1) as atp,
        ):
            qT_sb = st.tile([128, QPC], F16, tag="qT")
            ones_sb = st.tile([128, 1], BF16, tag="ones")
            mask_sb = st.tile([128, 16], F32, tag="mask")
            recip_sb = st.tile([128, QPC], F32, tag="recip")
            nc.vector.memset(ones_sb[:], 1.0)

            # ---- critical small loads first (sync queue order == byte order)
            nc.sync.dma_start(out=mask_sb[:], in_=maskb[:])

            es1 = ExitStack()  # SBUF transients: p1 + xts (freed before p34)
            p1 = es1.enter_context(tc.tile_pool(name="p1", bufs=1))
            wq_sb = p1.tile([128, D_MODEL], F16, tag="wq")
            nc.sync.dma_start(out=wq_sb[:], in_=wqr[:])
            wk_sb = p1.tile([128, D_MODEL], F16, tag="wk")
            nc.sync.dma_start(out=wk_sb[:], in_=wkr[:])
            xq_sb = p1.tile([128, NDM * QPC], F16, tag="xq")
            nc.sync.dma_start(out=xq_sb[:], in_=xqr[:])

            # ---- xT stream DMAs (one 3D-strided DMA per key group) ----
            xT3 = xT.rearrange("(i r) n -> r i n", r=128)  # [128, 16, 4096]
            xts = es1.enter_context(tc.tile_pool(name="xts", bufs=3))
            xts_t = []
            for kg in range(NKG):
                t = xts.tile([128, NDM * KG], F16, tag="xts", name=f"xts{kg}")
                nc.sync.dma_start(
                    out=t[:].rearrange("r (i n) -> r i n", n=KG),
                    in_=xT3[:, :, KG * kg : KG * (kg + 1)],
                )
                xts_t.append(t)

            # ---- xv loads (behind xT in the sync queues) ----
            xv_t = []
            for kt in range(NKT):
                t = xvp.tile([128, D_MODEL], BF16, tag="xv", name=f"xv{kt}")
                nc.sync.dma_start(out=t[:], in_=xv[128 * kt : 128 * (kt + 1), :])
                xv_t.append(t)

            # ---- qT projection ----
            with tc.tile_pool(name="psq", bufs=1, space="PSUM") as psqp:
                psq = psqp.tile([128, QPC], F32, tag="psq")
                for ic in range(NDM):
                    nc.tensor.matmul(
                        psq[:],
                        wq_sb[:, 128 * ic : 128 * (ic + 1)],
                        xq_sb[:, QPC * ic : QPC * (ic + 1)],
                        start=(ic == 0), stop=(ic == NDM - 1),
                    )
                nc.vector.tensor_copy(qT_sb[:], psq[:])

            # ---- fused pipeline: kT proj / scores / exp / denom / V[0:4] ----
            es2 = ExitStack()  # PSUM: psd + psv1 (freed before psv2)
            psdp = es2.enter_context(tc.tile_pool(name="psd", bufs=1, space="PSUM"))
            psv1 = es2.enter_context(tc.tile_pool(name="psv1", bufs=4, space="PSUM"))
            psd = psdp.tile([1, QPC], F32, tag="psd")
            pso1 = [
                psv1.tile([128, QPC], F32, tag="pso1", name=f"pso1_{j}")
                for j in range(4)
            ]
            at_t = []
            with (
                tc.tile_pool(name="psk", bufs=1, space="PSUM") as pskp,
                tc.tile_pool(name="pss", bufs=2, space="PSUM") as pssp,
            ):
                for kg in range(NKG):
                    psk = pskp.tile([128, KG], F32, tag="psk", name=f"psk{kg}")
                    for ic in range(NDM):
                        nc.tensor.matmul(
                            psk[:],
                            wk_sb[:, 128 * ic : 128 * (ic + 1)],
                            xts_t[kg][:, KG * ic : KG * (ic + 1)],
                            start=(ic == 0), stop=(ic == NDM - 1),
                        )
                    ktile = ktp.tile([128, KG], F16, tag="kt", name=f"kt{kg}")
                    nc.vector.tensor_copy(ktile[:], psk[:])

                    for sub in range(KG // 128):
                        kt = kg * (KG // 128) + sub
                        w = W[kt]
                        ps = pssp.tile([128, 512], F32, tag="pss", name=f"pss{kt}")
                        nc.tensor.matmul(
                            ps[:, :w],
                            ktile[:, 128 * sub : 128 * (sub + 1)],
                            qT_sb[:, QPC - w : QPC],
                            start=True, stop=True,
                        )
                        nc.vector.tensor_add(ps[:, :16], ps[:, :16], mask_sb[:])
                        at = atp.tile([128, w], BF16, tag=f"at{kt}")
                        nc.scalar.activation(
                            at[:], ps[:, :w], mybir.ActivationFunctionType.Exp
                        )
                        at_t.append(at)
                        nc.tensor.matmul(
                            psd[0:1, QPC - w : QPC],
                            ones_sb[:],
                            at[:],
                            start=(kt == 0), stop=(kt == NKT - 1),
                        )
                        # V matmul for output chunks 0-3, fused
                        for j in range(4):
                            nc.tensor.matmul(
                                pso1[j][:, QPC - w : QPC],
                                xv_t[kt][:, 128 * j : 128 * (j + 1)],
                                at[:],
                                start=(kt == 0), stop=(kt == NKT - 1),
                            )

            es1.close()  # free p1 + xts SBUF for the aoT / W2 pools

            # softmax denominators -> reciprocal, broadcast
            nc.vector.reciprocal(recip_sb[0:1, :], psd[0:1, :])
            nc.gpsimd.partition_broadcast(recip_sb[:], recip_sb[0:1, :])

            with tc.tile_pool(name="p34", bufs=1) as p34:
                ao_t = {}
                for j in range(4):
                    t = p34.tile([128, QPC], F16, tag=f"ao{j}")
                    # normalize here so fp16 stays in range
                    nc.vector.tensor_mul(t[:], pso1[j][:], recip_sb[:])
                    ao_t[j] = t
                es2.close()  # free psd + psv1 PSUM banks

                # ---- V matmul for output chunks 4-15 ----
                with tc.tile_pool(name="psv2", bufs=8, space="PSUM") as psv2:
                    for oc in range(4, NDM):
                        pso = psv2.tile(
                            [128, QPC], F32, tag="pso2", name=f"pso2_{oc}"
                        )
                        for kt in range(NKT):
                            w = W[kt]
                            nc.tensor.matmul(
                                pso[:, QPC - w : QPC],
                                xv_t[kt][:, 128 * oc : 128 * (oc + 1)],
                                at_t[kt][:],
                                start=(kt == 0), stop=(kt == NKT - 1),
                            )
                        t = p34.tile([128, QPC], F16, tag=f"ao{oc}")
                        nc.vector.tensor_mul(t[:], pso[:], recip_sb[:])
                        ao_t[oc] = t

                # ---- W2: outT = W2T.T @ attn_outT ----
                with (
                    tc.tile_pool(name="w2s", bufs=4) as w2s,
                    tc.tile_pool(name="outs", bufs=4) as outs,
                    tc.tile_pool(name="ps4", bufs=2, space="PSUM") as ps4,
                ):
                    for oc in range(NDM):
                        tw = w2s.tile([128, D_MODEL], F16, tag="w2")
                        nc.sync.dma_start(out=tw[:], in_=w2r[oc])
                        ps = ps4.tile([128, QPC], F32, tag="ps4")
                        for ic in range(NDM):
                            nc.tensor.matmul(
                                ps[:],
                                tw[:, 128 * ic : 128 * (ic + 1)],
                                ao_t[ic][:],
                                start=(ic == 0), stop=(ic == NDM - 1),
                            )
                        t = outs.tile([128, QPC], F32, tag="out")
                        nc.vector.tensor_copy(t[:], ps[:])
                        nc.sync.dma_start(
                            out=outT[128 * oc : 128 * (oc + 1), :], in_=t[:]
                        )

    nc.compile()
    return nc


def prepare_inputs(x, Wk, Wq, W2):
    """Host-side sharding/layout prep. Returns in_maps for the 8 cores."""
    x = np.asarray(x, dtype=np.float32)
    Wk = np.asarray(Wk, dtype=np.float32)
    Wq = np.asarray(Wq, dtype=np.float32)
    W2 = np.asarray(W2, dtype=np.float32)

    xT16 = np.ascontiguousarray(x.T).astype(np.float16)          # [D, N]
    xv16 = x.astype(ml_dtypes.bfloat16)                          # [N, D]

    def pack_chunks(aT, width):
        # aT [D_MODEL, width] -> [128, NDM*width]: out[r, width*ic + c] = aT[128ic+r, c]
        return np.ascontiguousarray(
            aT.reshape(NDM, 128, width).transpose(1, 0, 2).reshape(128, NDM * width)
        )

    wqr = pack_chunks(np.ascontiguousarray(Wq.T).astype(np.float16), D_HEAD)
    wkr = pack_chunks(np.ascontiguousarray(Wk.T).astype(np.float16), D_HEAD)
    # w2r[oc, r, 128*ic + o] = W2T[128ic+r, 128oc+o]
    w2T = np.ascontiguousarray(W2.T).astype(np.float16)
    w2r = np.ascontiguousarray(
        w2T.reshape(NDM, 128, NDM, 128).transpose(2, 1, 0, 3).reshape(NDM, 128, D_MODEL)
    )

    in_maps = []
    for c in range(NCORES):
        xqT = np.ascontiguousarray(x[c::NCORES].T).astype(np.float16)  # [D, QPC]
        xqr_c = pack_chunks(xqT, QPC)
        mask = np.zeros((128, 16), dtype=np.float32)
        j = np.arange(128)[:, None]
        t = np.arange(16)[None, :]
        mask[j > 8 * t + c] = MASK_NEG
        in_maps.append(
            {
                "xqr": xqr_c,
                "xT": xT16,
                "xv": xv16,
                "wqr": wqr,
                "wkr": wkr,
                "w2r": w2r,
                "maskb": mask,
            }
        )
    return in_maps


def assemble_output(results):
    res = np.stack([np.asarray(results[c]["outT"]) for c in range(NCORES)])
    # [c, d, m] -> out[8m+c, d]
    return np.ascontiguousarray(res.transpose(2, 0, 1).reshape(N_CTX, D_MODEL))


_CACHED = {}


def kernel(x, Wk, Wq, W2, _trace=False):
    if "nc" not in _CACHED:
        _CACHED["nc"] = build_program()
    nc = _CACHED["nc"]
    in_maps = prepare_inputs(x, Wk, Wq, W2)
    res = run_bass_kernel_spmd(nc, in_maps, core_ids=list(range(NCORES)), trace=_trace)
    out = assemble_output(res.results)
    if _trace:
        return out, res
    return out
